# revision 1
# baseline (speedup 1.0000x reference)
"""CodecAttention (sliding-window attention w/ QK-RMSNorm + ALiBi) on 8 trn2 cores.

Sharding: data-parallel over (batch, sequence-chunk): 2 batches x 4 chunks of 512
queries -> 8 cores. Each core recomputes K/V for its 512-token halo (zero-padded
for the first chunk), so there is no cross-core communication; the host only
slices/transposes inputs and concatenates the 8 disjoint output slices.

On-core pipeline (fp32 accumulation; matmul operands float32r by default,
KERNEL_PREC=bf16 selects a ~10% faster / less accurate variant):
  A) QKV projections from x^T (dim-on-partitions), producing q^T/k^T
     [qdim, tok] and V [tok, head, dh+1] (ones column yields softmax row
     sums during the AV matmul); QK RMSNorm over the flat 1024-dim axis via
     ACT-square + accumulating ones-matmul partition reduction, rsqrt as
     exp(-0.5*ln(x)) on ACT, 1/sqrt(dh) folded into the q scale.
  B) Head pairs: S^T tiles = k^T.T @ q^T (keys on partitions, two 3-bank
     PSUM halves per head), one fused DVE pass adds slope*dist + window
     mask (-1e30), ACT exp with a fixed offset (no row-max pass; scores
     are bounded), AV+rowsum with V-as-stationary, softmax denominators
     batched onto 8 partitions via tiny DMAs for one grouped reciprocal,
     K=1 ones-matmul partition broadcast, per-pair normalize multiply.
  C) out = attnT.T @ wo^T per token tile, streamed 512KB output DMAs.
"""

import contextlib
import ctypes
import os
import sys
import types

import ml_dtypes
import numpy as np

import concourse.bass as bass
import concourse.mybir as mybir
import concourse.tile as tile


def _install_axon_ntff_shim():
    """bass_utils' trace path wants antenv.axon_hooks, which this image lacks.
    Provide it, backed by direct ctypes calls into libaxon_pjrt.so (same ABI
    the agent boot would use). Degrades to hook=None if the .so is absent."""
    try:
        import antenv.axon_hooks  # noqa: F401
        return
    except ImportError:
        pass

    _hook_holder = [None]
    so_path = "/opt/axon/libaxon_pjrt.so"
    if os.path.exists(so_path):
        try:
            lib = ctypes.CDLL(so_path)
            if hasattr(lib, "axon_start_nrt_profile"):
                lib.axon_start_nrt_profile.argtypes = [
                    ctypes.POINTER(ctypes.c_int64), ctypes.c_size_t]
                lib.axon_start_nrt_profile.restype = ctypes.c_int64
                lib.axon_stop_nrt_profile.argtypes = [ctypes.c_char_p]
                lib.axon_stop_nrt_profile.restype = ctypes.c_int64

                @contextlib.contextmanager
                def _hook(output_dir, device_ids):
                    import jax
                    jax.devices()
                    if device_ids:
                        ids = (ctypes.c_int64 * len(device_ids))(*device_ids)
                        rc = lib.axon_start_nrt_profile(ids, len(device_ids))
                    else:
                        rc = lib.axon_start_nrt_profile(None, 0)
                    if rc != 0:
                        raise RuntimeError(f"axon_start_nrt_profile rc={rc}")
                    try:
                        yield
                    finally:
                        n = lib.axon_stop_nrt_profile(str(output_dir).encode())
                        if n < 0:
                            raise RuntimeError(f"axon_stop_nrt_profile rc={n}")

                _hook_holder[0] = _hook
        except OSError:
            pass

    mod = types.ModuleType("antenv.axon_hooks")
    mod.get_axon_ntff_profile_hook = lambda: _hook_holder[0]
    mod.set_axon_ntff_profile_hook = lambda h: _hook_holder.__setitem__(0, h)
    sys.modules["antenv.axon_hooks"] = mod


_install_axon_ntff_shim()

from concourse.bass_utils import run_bass_kernel_spmd  # noqa: E402
from bass_rust import ScopedClock  # noqa: E402

B, T, DIM = 2, 2048, 1024
H, DH, WINDOW = 16, 64, 512
P = 128
TQ = 512            # queries per core
TKV = 1024          # kv tokens per core (incl. 512 halo)
NCORES = 8
NQT = TQ // P       # 4
NKT = TKV // P      # 8
NDC = DIM // P      # 8
EXP_C = 10.0        # exp offset; true max masked score is ~6.0 for this data
F32 = mybir.dt.float32
F32R = mybir.dt.float32r
BF16 = mybir.dt.bfloat16
PREC = os.environ.get("KERNEL_PREC", "fp32r")
DT = BF16 if PREC == "bf16" else F32R
AF = mybir.ActivationFunctionType
ALU = mybir.AluOpType

SLOPES = [2.0 ** (-0.5 * (h + 1)) for h in range(H)]

# Score-tile packing: per head, scores are computed as S^T [key, query] in two
# 3-bank PSUM halves of [128, 1536]. Key-tile kt covers queries
# [KT_QLO[kt], KT_QLO[kt]+KT_W[kt]) at column offset KT_OFF[kt] of its half.
KT_W = [256, 256, 512, 512, 512, 512, 256, 256]
KT_OFF = [0, 256, 512, 1024, 0, 512, 1024, 1280]
KT_QLO = [0, 0, 0, 0, 0, 0, 256, 256]
HW_HALF = 1536
# 256-query output blocks: which key tiles contribute to each
QB_KTS = {0: [0, 1, 2, 3, 4, 5], 1: [2, 3, 4, 5, 6, 7]}
# merged AV matmul plan: (kt, out_col_lo, width, start, stop)
AV_PLAN = [
    (0, 0, 256, True, False),
    (1, 0, 256, False, False),
    (2, 0, 256, False, False), (2, 256, 256, True, False),
    (3, 0, 512, False, False),
    (4, 0, 512, False, False),
    (5, 0, 512, False, False),
    (6, 256, 256, False, False),
    (7, 256, 256, False, True),
]


class _SplitDrainTileContext(tile.TileContext):
    """The walrus build in this env rejects >1-2 sync-wait commands on one
    instruction; spread excess waits across same-engine NOPs placed directly
    before the over-limit instruction (per-engine program order preserved)."""

    def _split_excess_waits(self):
        nc = self.nc
        cur_list = nc.cur_bb.bb.instructions
        for blk in nc.m.functions[0].blocks:
            snapshot = list(blk.instructions)
            for inst in snapshot:
                si = inst.sync_info
                max_w = 1
                if si is None or len(si.on_wait) <= max_w:
                    continue
                waits = list(si.on_wait)
                si.on_wait = waits[:max_w]
                eng_obj = nc.engines[inst.engine]
                for w in waits[max_w:]:
                    nop_bi = eng_obj.nop(nofuse=True, hint="wait_split")
                    nop_inst = nop_bi.ins
                    nop_inst.sync_info = mybir.SyncInfo(on_wait=[w], on_update=[])
                    cur_list.remove(nop_inst)
                    blk.instructions.insert(
                        blk.instructions.index(inst), nop_inst)

    def _drain_and_barrier(self, tick_clock, wait_clock):
        self._split_excess_waits()
        drain_inst = self.nc.sync.drain()
        wait_clock.add_sem_waits(
            drain_inst.ins, ScopedClock({None: tick_clock.global_clock})
        )
        si = drain_inst.ins.sync_info
        if si is not None and len(si.on_wait) > 1:
            waits = list(si.on_wait)
            si.on_wait = waits[:1]
            for w in waits[1:]:
                nop = self.nc.sync.nop(nofuse=True, hint="drain_wait_split")
                nop.ins.sync_info = mybir.SyncInfo(on_wait=[w], on_update=[])
        self.nc.all_engine_barrier()
        assert self.sems is not None
        popped = self.nc._tile_sem_poison_stack.pop()
        assert popped is self._sem_poison
        self.nc.clear_and_free_semaphores(list(self.sems.allocated().values()))
        self.nc.all_engine_barrier()


def _src_nonce():
    import zlib
    with open(__file__, "rb") as f:
        return (zlib.crc32(f.read() + PREC.encode()) % 2048) + 8


def _build_program(debug=False):
    nc = bass.Bass()
    # dummy input whose shape changes with this file: busts HLO-keyed NEFF
    # caches (the BIR itself is not part of the HLO fingerprint)
    nonce = nc.declare_dram_parameter("nonce", [1, _src_nonce()], F32,
                                      isOutput=False)
    xT = nc.declare_dram_parameter("xT", [DIM, TKV], DT, isOutput=False)
    wT = nc.declare_dram_parameter("wT", [DIM, 3 * DIM], DT, isOutput=False)
    woT = nc.declare_dram_parameter("woT", [DIM, DIM], DT, isOutput=False)
    gam = nc.declare_dram_parameter("gam", [P, 2 * NDC], F32, isOutput=False)
    maskT = nc.declare_dram_parameter("maskT", [2, P, HW_HALF], F32, isOutput=False)
    out = nc.declare_dram_parameter("out", [TQ, DIM], F32, isOutput=True)
    if debug:
        qT_d = nc.declare_dram_parameter("qT_d", [P, NDC, TQ], DT, isOutput=True)
        kT_d = nc.declare_dram_parameter("kT_d", [P, NDC, TKV], DT, isOutput=True)
        V_d = nc.declare_dram_parameter("V_d", [P, NKT, H, DH + 1], DT, isOutput=True)
        aT_d = nc.declare_dram_parameter("aT_d", [P, NDC, TQ], DT, isOutput=True)
        pt_d = nc.declare_dram_parameter("pt_d", [2, P, 2, HW_HALF], DT, isOutput=True)

    with _SplitDrainTileContext(nc) as tc, \
            tc.tile_pool(name="persist", bufs=1) as pp, \
            tc.tile_pool(name="small", bufs=1) as psm:

        qT = pp.tile([P, NDC, TQ], DT, tag="qT")       # [p, odt, tok]
        kT = pp.tile([P, NDC, TKV], DT, tag="kT")
        V = pp.tile([P, NKT, H, DH + 1], DT, tag="V")  # [p=tok, kt, h, dh+ones]
        attnT = pp.tile([P, NDC, TQ], DT, tag="attnT")
        gam_sb = pp.tile([P, 2 * NDC], F32, tag="gam")
        ones_sb = pp.tile([P, 1], F32R, tag="ones")
        ones_row = pp.tile([1, P], F32, tag="onesrow")
        negc_sb = pp.tile([P, 1], F32, tag="negc")
        eps_sb = pp.tile([1, 1], F32, tag="eps")
        ln8_sb = pp.tile([1, 1], F32, tag="ln8")
        nc.sync.dma_start(gam_sb[:], gam[:])
        nc.vector.memset(ones_sb[:].bitcast(F32), 1.0)
        nc.vector.memset(ones_row[:], 1.0)
        nc.vector.memset(negc_sb[:], -EXP_C)
        nc.vector.memset(eps_sb[:], 1.0e-6)
        nc.vector.memset(ln8_sb[:], float(-0.5 * np.log(64.0)))
        nonce_sb = pp.tile([1, _src_nonce()], F32, tag="nonce")
        nc.sync.dma_start(nonce_sb[:], nonce[:])
        mask_sb = pp.tile([P, 2, HW_HALF], F32, tag="mask")
        nc.sync.dma_start(mask_sb[:], maskT.rearrange("h p w -> p h w"))
        ones_col = V[:, :, :, DH]
        nc.vector.memset(
            ones_col if DT == BF16 else ones_col.bitcast(F32), 1.0)

        # ---------------- Phase A: projections + RMSNorm ----------------
        with tc.tile_pool(name="xp", bufs=1) as px, \
                tc.tile_pool(name="wp", bufs=2) as pw, \
                tc.tile_pool(name="sqp", bufs=2) as psq, \
                tc.tile_pool(name="accp", bufs=1) as pacc, \
                tc.tile_pool(name="psA", bufs=4, space="PSUM") as psA, \
                tc.tile_pool(name="psS1", bufs=2, space="PSUM") as psS1, \
                tc.tile_pool(name="psBC", bufs=2, space="PSUM") as psBC:

            x_sb = px.tile([P, NDC, TKV], DT, tag="x")
            wq0_sb = pw.tile([P, NDC, 512], DT, tag="wslice", name="wq0")
            wq0_src = wT[:, 0:512].rearrange("(dc p) o -> p dc o", p=P)
            for dc in range(NDC):
                nc.sync.dma_start(wq0_sb[:, dc, :], wq0_src[:, dc, :])
                nc.sync.dma_start(x_sb[:, dc, :], xT[dc * P:(dc + 1) * P, :])

            # Q (tokens 512..1023 of the kv range) and K (all tokens)
            sqacc = {}
            for proj in range(2):
                dst = qT if proj == 0 else kT
                groups = [(TKV - TQ, 0)] if proj == 0 else [(0, 0), (512, 512)]
                for wh in range(2):
                    if proj == 0 and wh == 0:
                        w_sb = wq0_sb
                    else:
                        w_sb = pw.tile([P, NDC, 512], DT, tag="wslice")
                        nc.sync.dma_start(
                            w_sb[:],
                            wT[:, proj * DIM + wh * 512: proj * DIM + (wh + 1) * 512]
                            .rearrange("(dc p) o -> p dc o", p=P),
                        )
                    for ol in range(4):
                        odt = wh * 4 + ol
                        for (soff, doff) in groups:
                            ps = psA.tile([P, 512], F32, tag="projps")
                            for dc in range(NDC):
                                nc.tensor.matmul(
                                    ps[:],
                                    w_sb[:, dc, ol * P:(ol + 1) * P],
                                    x_sb[:, dc, soff:soff + 512],
                                    start=(dc == 0), stop=(dc == NDC - 1),
                                )
                            nc.scalar.copy(dst[:, odt, doff:doff + 512], ps[:])
                            sq = psq.tile([P, 512], F32, tag="sq")
                            nc.scalar.activation(sq[:], ps[:], AF.Square)
                            key = (proj, doff)
                            if odt == 0:
                                acc = pacc.tile([P, 512], F32,
                                                tag=f"acc{proj}_{doff}",
                                                name="acc")
                                sqacc[key] = acc
                                nc.vector.tensor_copy(acc[:], sq[:])
                            else:
                                nc.vector.tensor_add(sqacc[key][:],
                                                     sqacc[key][:], sq[:])

            # rsqrt(mean + eps) per token, Newton-refined; broadcast to 128 parts
            # V projection: [tok, head, dh]; vh innermost so consecutive
            # matmuls share the x-chunk stationary operand (ldw-opt dedups)
            wv_sb = []
            for vh in range(2):
                w_sb = pw.tile([P, NDC, 512], DT, tag="wslice")
                nc.sync.dma_start(
                    w_sb[:],
                    wT[:, 2 * DIM + vh * 512: 2 * DIM + (vh + 1) * 512]
                    .rearrange("(dc p) o -> p dc o", p=P),
                )
                wv_sb.append(w_sb)
            for tt in range(NKT):
                pss = [psA.tile([P, 512], F32, tag="projps", name="psv")
                       for _ in range(2)]
                for dc in range(NDC):
                    for vh in range(2):
                        nc.tensor.matmul(
                            pss[vh][:],
                            x_sb[:, dc, tt * P:(tt + 1) * P],
                            wv_sb[vh][:, dc, :],
                            start=(dc == 0), stop=(dc == NDC - 1),
                        )
                for vh in range(2):
                    nc.scalar.copy(
                        V[:, tt, vh * 8:(vh + 1) * 8, :DH],
                        pss[vh][:].rearrange("p (h c) -> p h c", c=DH),
                    )

            # rsqrt(mean+eps) = exp(-0.5*ln(ss/DIM + eps)); the 1/sqrt(dh)
            # score scale folds into the exp bias for q
            bcasts = {}
            for (proj, doff), acc in sqacc.items():
                ss = psS1.tile([1, 512], F32, tag="ssq", name="ssq")
                nc.tensor.matmul(ss[:], ones_sb[:].bitcast(F32), acc[:],
                                 start=True, stop=True)
                a = psm.tile([1, 512], F32, tag="a")
                nc.scalar.activation(a[:], ss[:], AF.Ln,
                                     bias=eps_sb[:], scale=1.0 / DIM)
                y = psm.tile([1, 512], F32, tag="y")
                nc.scalar.activation(y[:], a[:], AF.Exp,
                                     bias=(ln8_sb[:] if proj == 0 else 0.0),
                                     scale=-0.5)
                # broadcast over partitions via K=1 ones-matmul (plain fp32)
                bc = psBC.tile([P, 512], F32, tag="bc", name="bc")
                nc.tensor.matmul(bc[:], ones_row[:], y[:], start=True, stop=True)
                bcasts[(proj, doff)] = bc

            # normalize in place (x gamma)
            for proj in range(2):
                dst = qT if proj == 0 else kT
                for odt in range(NDC):
                    gap = gam_sb[:, proj * NDC + odt: proj * NDC + odt + 1]
                    for doff in ([0] if proj == 0 else [0, 512]):
                        sl = dst[:, odt, doff:doff + 512]
                        nc.vector.scalar_tensor_tensor(
                            sl, sl, gap, bcasts[(proj, doff)][:],
                            op0=ALU.mult, op1=ALU.mult,
                        )

        if debug:
            nc.sync.dma_start(qT_d[:], qT[:])
            nc.sync.dma_start(kT_d[:], kT[:])
            nc.sync.dma_start(V_d[:], V[:])

        # ---------------- Phase B: attention (head pairs) ----------------
        with tc.tile_pool(name="maskp", bufs=1) as pm, \
                tc.tile_pool(name="ptp", bufs=2) as ppt, \
                tc.tile_pool(name="rnp", bufs=1) as prn, \
                tc.tile_pool(name="psS", bufs=1, space="PSUM") as psS, \
                tc.tile_pool(name="psO", bufs=1, space="PSUM") as psO:

            s16 = [pm.tile([8, TQ], F32, tag="s16a", name="s16a"),
                   pm.tile([8, TQ], F32, tag="s16b", name="s16b")]

            for hp in range(NDC):
                ps_o = [psO.tile([DH + 1, TQ], F32, tag=f"avps{hi}", name=f"avps{hi}")
                        for hi in range(2)]
                pts = {0: [], 1: []}
                for half in range(2):
                    ps_pair = [psS.tile([P, HW_HALF], F32, tag=f"sps{hi}", name=f"sps{hi}")
                               for hi in range(2)]
                    for ktl in range(4):
                        kt = half * 4 + ktl
                        off, wdt, qlo = KT_OFF[kt], KT_W[kt], KT_QLO[kt]
                        for hi in range(2):
                            po = DH * hi
                            nc.tensor.matmul(
                                ps_pair[hi][:, off:off + wdt],
                                kT[po:po + DH, hp, kt * P:(kt + 1) * P],
                                qT[po:po + DH, hp, qlo:qlo + wdt],
                                start=True, stop=True,
                            )
                    for hi in range(2):
                        h = 2 * hp + hi
                        nc.vector.scalar_tensor_tensor(
                            ps_pair[hi][:], mask_sb[:, half], SLOPES[h],
                            ps_pair[hi][:], op0=ALU.mult, op1=ALU.add,
                        )
                        pt = ppt.tile([P, HW_HALF], DT, tag=f"pt{hi}")
                        nc.scalar.activation(pt[:], ps_pair[hi][:], AF.Exp,
                                             bias=negc_sb[:])
                        if debug and hp == 0:
                            nc.sync.dma_start(pt_d[hi, :, half, :], pt[:])
                        pts[hi].append(pt)

                for hi in range(2):
                    h = 2 * hp + hi
                    po = DH * hi
                    for qb in range(2):
                        kts = QB_KTS[qb]
                        for i, kt in enumerate(kts):
                            half, off, qlo = kt // 4, KT_OFF[kt], KT_QLO[kt]
                            c0 = off + qb * 256 - qlo
                            nc.tensor.matmul(
                                ps_o[hi][:, qb * 256:(qb + 1) * 256],
                                V[:, kt, h, :],
                                pts[hi][half][:, c0:c0 + 256],
                                start=(i == 0), stop=(i == len(kts) - 1),
                            )
                    # stash the softmax denominator row; head h -> partition
                    # h of s16 via a tiny SBUF->SBUF DMA (engines cannot write
                    # partition offsets other than 0/32/64)
                    r = prn.tile([1, TQ], F32, tag=f"r{hi}", name="r")
                    nc.vector.tensor_copy(r[:], ps_o[hi][DH:DH + 1, :])
                    nc.sync.dma_start(s16[h // 8][h % 8:h % 8 + 1, :], r[:])
                    nc.vector.tensor_copy(attnT[po:po + DH, hp, :], ps_o[hi][:DH, :])

                if hp % 4 == 3:
                    # normalize the finished half (8 heads): batched reciprocal
                    # on 8 partitions, DMA back to partition 0 for K=1 bcasts
                    g = hp // 4
                    rc8 = pm.tile([8, TQ], F32, tag=f"rc{g}", name="rc8")
                    nc.vector.reciprocal(rc8[:], s16[g][:])
                    rcf = pm.tile([1, 8, TQ], F32, tag=f"rcf{g}", name="rcf")
                    nc.sync.dma_start(rcf[:], rc8[:])
                    for hp2 in range(4 * g, 4 * g + 4):
                        rb = psO.tile([P, TQ], F32,
                                      tag=f"avps{hp2 % 2}", name="rb")
                        for hi2 in range(2):
                            nc.tensor.matmul(
                                rb[DH * hi2:DH * (hi2 + 1), :],
                                ones_row[:, :DH],
                                rcf[0:1, 2 * hp2 + hi2 - 8 * g, :],
                                start=True, stop=True)
                        nc.vector.tensor_mul(attnT[:, hp2, :],
                                             attnT[:, hp2, :], rb[:])


        if debug:
            nc.sync.dma_start(aT_d[:], attnT[:])

        # ---------------- Phase C: output projection ----------------
        with tc.tile_pool(name="wop", bufs=2) as pwo, \
                tc.tile_pool(name="outp", bufs=1) as pout, \
                tc.tile_pool(name="psC", bufs=3, space="PSUM") as psC:
            out_sb = pout.tile([P, NQT, DIM], F32, tag="out")
            wo_sbs = []
            for oh in range(2):
                w_sb = pwo.tile([P, NDC, 512], DT, tag="wo")
                nc.sync.dma_start(
                    w_sb[:],
                    woT[:, oh * 512:(oh + 1) * 512]
                    .rearrange("(adc p) o -> p adc o", p=P),
                )
                wo_sbs.append(w_sb)
            out_r = out.rearrange("(tt p) o -> p tt o", p=P)
            for tt in range(NQT):
                pss = [psC.tile([P, 512], F32, tag="cps", name="psc")
                       for _ in range(2)]
                for adc in range(NDC):
                    for oh in range(2):
                        nc.tensor.matmul(
                            pss[oh][:],
                            attnT[:, adc, tt * P:(tt + 1) * P],
                            wo_sbs[oh][:, adc, :],
                            start=(adc == 0), stop=(adc == NDC - 1),
                        )
                for oh in range(2):
                    nc.vector.tensor_copy(
                        out_sb[:, tt, oh * 512:(oh + 1) * 512], pss[oh][:])
                nc.sync.dma_start(out_r[:, tt, :], out_sb[:, tt, :])

    return nc


def _build_mask(chunk0: bool) -> np.ndarray:
    m = np.full((2, P, HW_HALF), -1e30, np.float32)
    for kt in range(NKT):
        half, off, w, qlo = kt // 4, KT_OFF[kt], KT_W[kt], KT_QLO[kt]
        kl = np.arange(P)[:, None]
        qg = (qlo + np.arange(w))[None, :]
        kv = kt * P + kl
        dist = kv - (qg + 512)          # j - i in global coords
        valid = (dist <= 0) & (dist >= -WINDOW)
        if chunk0:
            valid &= kv >= 512
        m[half, :, off:off + w] = np.where(valid, dist, -1e30).astype(np.float32)
    return m


_NC = None
LAST = None  # BassKernelResults of the most recent run (exec_time_ns when traced)


def _get_nc():
    global _NC
    if _NC is None:
        _NC = _build_program()
    return _NC


def kernel(x, wq, wk, wv, wo, q_gamma, k_gamma):
    x = np.ascontiguousarray(np.asarray(x, np.float32))
    wq = np.asarray(wq, np.float32)
    wk = np.asarray(wk, np.float32)
    wv = np.asarray(wv, np.float32)
    wo = np.asarray(wo, np.float32)
    q_gamma = np.asarray(q_gamma, np.float32)
    k_gamma = np.asarray(k_gamma, np.float32)

    np_dt = ml_dtypes.bfloat16 if PREC == "bf16" else np.float32
    wT_host = np.ascontiguousarray(
        np.concatenate([wq.T, wk.T, wv.T], axis=1).astype(np_dt))
    woT_host = np.ascontiguousarray(wo.T.astype(np_dt))
    gam_host = np.ascontiguousarray(np.concatenate(
        [q_gamma.reshape(NDC, P).T, k_gamma.reshape(NDC, P).T], axis=1))
    mask_c0 = _build_mask(True)
    mask_ci = _build_mask(False)

    in_maps = []
    for c in range(NCORES):
        b, j = divmod(c, 4)
        lo = j * TQ - WINDOW
        xs = x[b, max(0, lo): j * TQ + TQ, :]
        if lo < 0:
            xs = np.concatenate(
                [np.zeros((-lo, DIM), np.float32), xs], axis=0)
        in_maps.append({
            "nonce": np.zeros((1, _src_nonce()), np.float32),
            "xT": np.ascontiguousarray(xs.T.astype(np_dt)),
            "wT": wT_host,
            "woT": woT_host,
            "gam": gam_host,
            "maskT": mask_c0 if j == 0 else mask_ci,
        })

    global LAST
    trace = bool(int(os.environ.get("KERNEL_TRACE", "0") or 0))
    try:
        LAST = run_bass_kernel_spmd(
            _get_nc(), in_maps, list(range(NCORES)), trace=trace)
    except Exception:
        # a previously-wedged device surfaces as NRT_EXEC_UNIT_UNRECOVERABLE
        # on the first touch; reset the accelerator once and retry
        try:
            lib = ctypes.CDLL("/opt/axon/libaxon_pjrt.so")
            lib.axon_reset.restype = ctypes.c_int64
            import jax
            jax.devices()
            lib.axon_reset()
        except Exception:
            pass
        LAST = run_bass_kernel_spmd(
            _get_nc(), in_maps, list(range(NCORES)), trace=trace)

    full = np.empty((B, T, DIM), np.float32)
    for c in range(NCORES):
        b, j = divmod(c, 4)
        full[b, j * TQ:(j + 1) * TQ, :] = LAST.results[c]["out"]
    return full

